# revision 1
# baseline (speedup 1.0000x reference)
"""Trainium2 Bass kernel for nn_PolicyNetwork (GRU + MLP head).

Strategy: data-parallel over batch (B=256 -> 32 per core, 8 cores).
All on-device tensors use the "transposed" layout [feature, batch] so the
512-step GRU recurrence needs no per-step transposes:
  - x is transposed on the host (xT [D, T*BS] bf16) so phase 1 uses plain
    contiguous DMA loads (no on-device DMA-transpose).
  - x_proj precompute: x_projT[g, t*b] = W_ih @ x^T, written to DRAM
    scratch, biases folded in.
  - recurrence: h_projT = W_hh^T-stationary matmuls streaming hT (N=32),
    W_hh in fp8e4m3 (x16 scale), gates on DVE/ACT in [128, chunk, 32]
    tiles, h kept as bf16. The per-step rate is bound by the PE
    LDWEIGHTS+MATMUL issue rate (~73ns/pair x 48 pairs).
  - head: two small W-stationary matmuls + tanh.
Host side: weights pre-transposed + cast, biases pre-summed/reshaped,
output gathered and transposed back.
"""

import numpy as np
import ml_dtypes
from contextlib import ExitStack

import concourse.bass as bass
import concourse.bacc as bacc
import concourse.tile as tile
import concourse.mybir as mybir
from concourse.bass_utils import run_bass_kernel_spmd

T, B, D, H, M, A = 512, 256, 256, 512, 512, 64
NCORES = 8
BS = B // NCORES          # 32 batch per core
G = 3 * H                 # 1536 gate width
MCH = G // 128            # 12 gate chunks
KH = H // 128             # 4 hidden chunks
KD = D // 128             # 2 input chunks
MH = M // 128             # 4 mlp chunks
TBC = 512                 # tb-chunk size for x_proj
PREF = 16                 # steps per xp prefetch batch (PREF*BS == TBC)

f32 = mybir.dt.float32
bf16 = mybir.dt.bfloat16
AF = mybir.ActivationFunctionType
bf16_np = ml_dtypes.bfloat16


WHH_FP8 = True          # W_hh fp8e4m3 x16 (halves exposed LDWEIGHTS time)
FP8_SCALE = 16.0

def build(nsteps: int = T, rec_mult: int = 1):
    nc = bacc.Bacc("TRN2", target_bir_lowering=False, debug=False)
    tbn = nsteps * BS
    wdt = mybir.dt.float8e4 if WHH_FP8 else bf16

    xTd = nc.dram_tensor("xT", [D, tbn], bf16, kind="ExternalInput").ap()
    wihT = nc.dram_tensor("wihT", [D, G], bf16, kind="ExternalInput").ap()
    whhT = nc.dram_tensor("whhT", [H, G], wdt, kind="ExternalInput").ap()
    w1T = nc.dram_tensor("w1T", [H, M], bf16, kind="ExternalInput").ap()
    w2T = nc.dram_tensor("w2T", [M, A], bf16, kind="ExternalInput").ap()
    bsum = nc.dram_tensor("bsum", [128, MCH], f32, kind="ExternalInput").ap()
    b1T = nc.dram_tensor("b1T", [128, MH], f32, kind="ExternalInput").ap()
    b2c = nc.dram_tensor("b2c", [A, 1], f32, kind="ExternalInput").ap()
    outT = nc.dram_tensor("outT", [A, BS], f32, kind="ExternalOutput").ap()
    # x_projT scratch: [m-chunk, partition, t*b] so per-chunk reads/writes are
    # contiguous along tb.
    xpT = nc.dram_tensor("xpT", [MCH, 128, tbn], bf16).ap()

    nchunks = tbn // TBC

    with tile.TileContext(nc) as tc, ExitStack() as ctx:
        wpool = ctx.enter_context(tc.tile_pool(name="weights", bufs=1))

        wih_sb = wpool.tile([128, KD, G], bf16, tag="wih")
        for k in range(KD):
            nc.sync.dma_start(wih_sb[:, k, :], wihT[k * 128:(k + 1) * 128, :])
        whh_sb = wpool.tile([128, KH, G], wdt, tag="whh")
        for k in range(KH):
            nc.sync.dma_start(whh_sb[:, k, :], whhT[k * 128:(k + 1) * 128, :])
        w1_sb = wpool.tile([128, KH, M], bf16, tag="w1")
        for k in range(KH):
            nc.sync.dma_start(w1_sb[:, k, :], w1T[k * 128:(k + 1) * 128, :])
        w2_sb = wpool.tile([128, MH, A], bf16, tag="w2")
        for k in range(MH):
            nc.sync.dma_start(w2_sb[:, k, :], w2T[k * 128:(k + 1) * 128, :])
        bsum_sb = wpool.tile([128, MCH], f32, tag="bsum")
        nc.sync.dma_start(bsum_sb[:], bsum[:, :])
        b1_sb = wpool.tile([128, MH], f32, tag="b1")
        nc.sync.dma_start(b1_sb[:], b1T[:, :])
        b2_sb = wpool.tile([A, 1], f32, tag="b2")
        nc.sync.dma_start(b2_sb[:], b2c[:, :])

        # ---- Phase 1: x_projT = W_ih @ x^T + (b_ih + b_hh), to DRAM scratch.
        # xpT row order is permuted so half-p gate slices are contiguous:
        # rows = [r0,r1,z0,z1, r2,r3,z2,z3, n0,n1,n2,n3] (gate chunk m -> POS[m])
        POS = {0: 0, 1: 1, 4: 2, 5: 3, 2: 4, 3: 5, 6: 6, 7: 7,
               8: 8, 9: 9, 10: 10, 11: 11}
        with tc.tile_pool(name="xproj", bufs=3) as xpool, \
             tc.tile_pool(name="xproj_psum", bufs=4, space="PSUM") as xppsum:
            for c in range(nchunks):
                xT = xpool.tile([128, KD, TBC], bf16, tag="xT")
                for k in range(KD):
                    nc.sync.dma_start(
                        xT[:, k, :],
                        xTd[k * 128:(k + 1) * 128, c * TBC:(c + 1) * TBC],
                    )
                for m in range(MCH):
                    ps = xppsum.tile([128, TBC], f32, tag="p512")
                    for k in range(KD):
                        nc.tensor.matmul(
                            ps[:],
                            wih_sb[:, k, m * 128:(m + 1) * 128],
                            xT[:, k, :],
                            start=(k == 0),
                            stop=(k == KD - 1),
                        )
                    xp = xpool.tile([128, TBC], bf16, tag="xp")
                    if m % 2 == 0:
                        nc.scalar.activation(
                            xp[:], ps[:], AF.Identity, bias=bsum_sb[:, m:m + 1]
                        )
                    else:
                        nc.vector.tensor_scalar_add(
                            xp[:], ps[:], bsum_sb[:, m:m + 1]
                        )
                    nc.sync.dma_start(
                        xpT[POS[m], :, c * TBC:(c + 1) * TBC], xp[:]
                    )

        # ---- Phase 2: GRU recurrence over nsteps, half-split pipeline.
        # Half p covers H-chunks {2p, 2p+1}. Gate-chunk groups for half p:
        # rz rows 4p..4p+4 of xpT order (= r_{2p}, r_{2p+1}, z_{2p}, z_{2p+1}),
        # n rows 8+2p..8+2p+2. MM emission: [h0g k01][h1g k01][h0g k23][h1g k23]
        # so PE(t+1) can start on half-0 of h_{t+1} while gates of half 1 run.
        hpool = ctx.enter_context(tc.tile_pool(name="h", bufs=3))
        gpool = ctx.enter_context(tc.tile_pool(name="gates", bufs=2))
        xbpool = ctx.enter_context(tc.tile_pool(name="xpbuf", bufs=2))
        rpsum = ctx.enter_context(tc.tile_pool(name="rec_psum", bufs=2, space="PSUM"))

        # gate-chunk m (natural order) for half p: rz groups and n groups
        RZ_M = [[0, 1, 4, 5], [2, 3, 6, 7]]   # natural m for prz[p] rows 0..3
        N_M = [[8, 9], [10, 11]]

        h_init0 = hpool.tile([128, 2, BS], bf16, tag="h0")
        h_init1 = hpool.tile([128, 2, BS], bf16, tag="h1")
        h_cur = [h_init0, h_init1]
        nc.vector.memset(h_cur[0][:], 0.0)
        nc.vector.memset(h_cur[1][:], 0.0)

        def rhs_h(k):
            return h_cur[k // 2][:, k % 2, :]

        xp_buf = None
        for tt in range(nsteps * rec_mult):
            t = tt % nsteps
            bi = t % PREF
            if bi == 0:
                xp_buf = xbpool.tile([128, MCH, TBC], bf16, tag="xpbuf")
                for m in range(MCH):
                    nc.sync.dma_start(
                        xp_buf[:, m, :], xpT[m, :, t * BS:t * BS + TBC]
                    )
            xs = slice(bi * BS, (bi + 1) * BS)

            prz0 = rpsum.tile([128, 4, BS], f32, tag="prz0")
            prz1 = rpsum.tile([128, 4, BS], f32, tag="prz1")
            pn0 = rpsum.tile([128, 2, BS], f32, tag="pn0")
            pn1 = rpsum.tile([128, 2, BS], f32, tag="pn1")
            prz = [prz0, prz1]
            pn = [pn0, pn1]

            # One accumulation group per PSUM bank (start=True zeroes the whole
            # 2KB bank): start on the first MM into the tile, stop on the last.
            def mm_batch(p, ks):
                first = ks[0] == 0
                last = ks[-1] == KH - 1
                for i, m in enumerate(RZ_M[p]):
                    for k in ks:
                        nc.tensor.matmul(
                            prz[p][:, i, :],
                            whh_sb[:, k, m * 128:(m + 1) * 128],
                            rhs_h(k),
                            start=(first and i == 0 and k == ks[0]),
                            stop=(last and i == len(RZ_M[p]) - 1 and k == ks[-1]),
                        )
                for i, m in enumerate(N_M[p]):
                    for k in ks:
                        nc.tensor.matmul(
                            pn[p][:, i, :],
                            whh_sb[:, k, m * 128:(m + 1) * 128],
                            rhs_h(k),
                            start=(first and i == 0 and k == ks[0]),
                            stop=(last and i == len(N_M[p]) - 1 and k == ks[-1]),
                        )

            # Emit both k-halves of half-0 first so prz0/pn0 complete
            # after 24 MMs (not 36): h0' is ready ~12 pairs earlier, which
            # keeps next step's first MM batch from stalling on the gate
            # chain. Steady state: P >= max(48r, 24r + L) for both halves.
            h_new = [None, None]
            inv = 1.0 / FP8_SCALE if WHH_FP8 else 1.0

            def emit_gates(p):
                # a = psum*inv + xp; r,z = sig(a); zbar = sig(-a_z).
                # h' = z*h + zbar*n: m1 = z*h runs early (off the tanh
                # path), so the post-tanh tail is just m2, hn.
                a = gpool.tile([128, 4, BS], bf16, tag=f"rzp{p}",
                               name=f"a{p}")
                if WHH_FP8:
                    nc.vector.scalar_tensor_tensor(
                        a[:], prz[p][:], inv, xp_buf[:, 4 * p:4 * p + 4, xs],
                        op0=mybir.AluOpType.mult, op1=mybir.AluOpType.add)
                else:
                    nc.vector.tensor_add(
                        a[:], prz[p][:], xp_buf[:, 4 * p:4 * p + 4, xs])
                rz = gpool.tile([128, 4, BS], bf16, tag=f"rz{p}",
                                name=f"rz{p}")
                nc.scalar.activation(rz[:], a[:], AF.Sigmoid)
                t1 = gpool.tile([128, 2, BS], bf16, tag=f"t1{p}",
                                name=f"t1{p}")
                if WHH_FP8:
                    nc.vector.scalar_tensor_tensor(
                        t1[:], pn[p][:], inv, rz[:, 0:2, :],
                        op0=mybir.AluOpType.mult, op1=mybir.AluOpType.mult)
                else:
                    nc.vector.tensor_mul(t1[:], pn[p][:], rz[:, 0:2, :])
                t2 = gpool.tile([128, 2, BS], bf16, tag=f"t2{p}",
                                name=f"t2{p}")
                nc.vector.tensor_add(
                    t2[:], t1[:], xp_buf[:, 8 + 2 * p:8 + 2 * p + 2, xs])
                n_g = gpool.tile([128, 2, BS], bf16, tag=f"n{p}",
                                 name=f"n{p}")
                nc.scalar.activation(n_g[:], t2[:], AF.Tanh)
                zbar = gpool.tile([128, 2, BS], bf16, tag=f"zb{p}",
                                  name=f"zb{p}")
                nc.scalar.activation(zbar[:], a[:, 2:4, :], AF.Sigmoid,
                                     scale=-1.0)
                m1 = gpool.tile([128, 2, BS], bf16, tag=f"m1{p}",
                                name=f"m1{p}")
                nc.vector.tensor_mul(m1[:], rz[:, 2:4, :], h_cur[p][:])
                m2 = gpool.tile([128, 2, BS], bf16, tag=f"m2{p}",
                                name=f"m2{p}")
                nc.vector.tensor_mul(m2[:], zbar[:], n_g[:])
                hn = hpool.tile([128, 2, BS], bf16, tag=f"h{p}",
                                name=f"hn{p}")
                nc.vector.tensor_add(hn[:], m1[:], m2[:])
                h_new[p] = hn

            # half-0 MMs, then its gates (emission order = scheduler
            # priority), then half-1 MMs and gates.
            mm_batch(0, (0, 1))
            mm_batch(0, (2, 3))
            emit_gates(0)
            mm_batch(1, (0, 1))
            mm_batch(1, (2, 3))
            emit_gates(1)
            h_cur = h_new

        # ---- Phase 3: MLP head
        ps_hid = rpsum.tile([128, MH, BS], f32, tag="prz0")
        for mh in range(MH):
            for k in range(KH):
                nc.tensor.matmul(
                    ps_hid[:, mh, :],
                    w1_sb[:, k, mh * 128:(mh + 1) * 128],
                    rhs_h(k),
                    start=(mh == 0 and k == 0),
                    stop=(mh == MH - 1 and k == KH - 1),
                )
        hid = gpool.tile([128, MH, BS], bf16, tag="hid")
        for mh in range(MH):
            nc.scalar.activation(
                hid[:, mh, :], ps_hid[:, mh, :], AF.Tanh, bias=b1_sb[:, mh:mh + 1]
            )
        ps_act = rpsum.tile([A, BS], f32, tag="pn0")
        for k in range(MH):
            nc.tensor.matmul(
                ps_act[:],
                w2_sb[:, k, :],
                hid[:, k, :],
                start=(k == 0),
                stop=(k == MH - 1),
            )
        act = gpool.tile([A, BS], f32, tag="act")
        nc.scalar.activation(act[:], ps_act[:], AF.Tanh, bias=b2_sb[:, 0:1])
        nc.sync.dma_start(outT[:, :], act[:])

    nc.compile()
    return nc


def prep_inputs(x, W_ih, W_hh, b_ih, b_hh, W1, b1, W2, b2, nsteps: int = T):
    """Host-side prep: transpose/cast weights, shard x over batch."""
    x = np.asarray(x, dtype=np.float32)[:nsteps]
    whh_t = np.ascontiguousarray(np.asarray(W_hh, np.float32).T)
    if WHH_FP8:
        whh_in = (whh_t * FP8_SCALE).astype(ml_dtypes.float8_e4m3)
    else:
        whh_in = whh_t.astype(bf16_np)
    common = {
        "wihT": np.ascontiguousarray(np.asarray(W_ih).T).astype(bf16_np),
        "whhT": whh_in,
        "w1T": np.ascontiguousarray(np.asarray(W1).T).astype(bf16_np),
        "w2T": np.ascontiguousarray(np.asarray(W2).T).astype(bf16_np),
        "bsum": np.ascontiguousarray(
            (np.asarray(b_ih, np.float32) + np.asarray(b_hh, np.float32))
            .reshape(MCH, 128).T),
        "b1T": np.ascontiguousarray(np.asarray(b1, np.float32).reshape(MH, 128).T),
        "b2c": np.ascontiguousarray(np.asarray(b2, np.float32).reshape(A, 1)),
    }
    in_maps = []
    for i in range(NCORES):
        shard = x[:, i * BS:(i + 1) * BS, :].reshape(nsteps * BS, D)
        m = dict(common)
        m["xT"] = np.ascontiguousarray(shard.astype(bf16_np).T)
        in_maps.append(m)
    return in_maps


_CACHE = {}


def run(inputs: dict, nsteps: int = T, trace: bool = False):
    key = nsteps
    if key not in _CACHE:
        _CACHE[key] = build(nsteps)
    nc = _CACHE[key]
    in_maps = prep_inputs(**inputs, nsteps=nsteps)
    res = run_bass_kernel_spmd(
        nc, in_maps, core_ids=list(range(NCORES)), trace=trace
    )
    outs = [r["outT"] for r in res.results]
    full = np.concatenate([o.T for o in outs], axis=0)
    return full.astype(np.float32), res


def kernel(**inputs) -> np.ndarray:
    out, _ = run(inputs)
    return out



# revision 2
# speedup vs baseline: 2.5178x; 2.5178x over previous
"""Trainium2 Bass kernel v3: single-pipe recurrence with half-split h
production (baseline-style cross-step overlap), kernel2's shortened gate chain
(identity-MM xp injection into PSUM, psum-direct sigmoid, zbar/m1 off the
critical path), and phase-1 fused into the step loop (x_proj SBUF-resident).

Per step: psum accumulated k-batch-major (k0..k3) so the last-ready h half
gates only the k2/k3 batches; within each k batch the rz chunks of half 0
come first so sigmoid-0 can fire before the batch finishes. Gate chain per
half: sig(psum_rz/16) -> t1=psum_n*r -> t2=t1+xp_n -> tanh(t2/16) -> m2 ->
h'. Same host-side input contract as kernel2.
"""

import numpy as np
import ml_dtypes
from contextlib import ExitStack

import concourse.bass as bass
import concourse.bacc as bacc
import concourse.tile as tile
import concourse.mybir as mybir
from concourse.bass_utils import run_bass_kernel_spmd


T, B, D, H, M, A = 512, 256, 256, 512, 512, 64
NCORES = 8
BS = B // NCORES          # 32
G = 3 * H
MCH = G // 128            # 12 (r: 0-3, z: 4-7, n: 8-11)
NRZ = 8
NN = 4
KH = H // 128
KD = D // 128
MH = M // 128
CHUNK = 16
TBC = CHUNK * BS

f32 = mybir.dt.float32
bf16 = mybir.dt.bfloat16
fp8 = mybir.dt.float8e4
AF = mybir.ActivationFunctionType
ALU = mybir.AluOpType
bf16_np = ml_dtypes.bfloat16
fp8_np = ml_dtypes.float8_e4m3

S = 16.0
INV = 1.0 / S

# gate-chunk indices per h-half p: r chunks, z chunks, n chunks
RZ_M = {p: [2 * p, 2 * p + 1, 4 + 2 * p, 5 + 2 * p] for p in (0, 1)}
N_M = {p: [8 + 2 * p, 9 + 2 * p] for p in (0, 1)}


def build(nsteps: int = T, rec_mult: int = 1):
    assert nsteps % CHUNK == 0
    nc = bacc.Bacc("TRN2", target_bir_lowering=False, debug=False)
    tbn = nsteps * BS
    nchunks = nsteps // CHUNK

    xTd = nc.dram_tensor("xT", [D, tbn], bf16, kind="ExternalInput").ap()
    wihT = nc.dram_tensor("wihT", [D, G], bf16, kind="ExternalInput").ap()
    whhT = nc.dram_tensor("whhT", [H, G], fp8, kind="ExternalInput").ap()
    identd = nc.dram_tensor("identd", [128, 128], fp8, kind="ExternalInput").ap()
    bhnd = nc.dram_tensor("bhnd", [128, NN, BS], bf16, kind="ExternalInput").ap()
    p1bd = nc.dram_tensor("p1bd", [128, MCH], f32, kind="ExternalInput").ap()
    w1T = nc.dram_tensor("w1T", [H, M], bf16, kind="ExternalInput").ap()
    b1T = nc.dram_tensor("b1T", [128, MH], f32, kind="ExternalInput").ap()
    w2T = nc.dram_tensor("w2T", [M, A], bf16, kind="ExternalInput").ap()
    b2c = nc.dram_tensor("b2c", [A, 1], f32, kind="ExternalInput").ap()
    outT = nc.dram_tensor("outT", [A, BS], f32, kind="ExternalOutput").ap()

    with tile.TileContext(nc) as tc, ExitStack() as ctx:
        wpool = ctx.enter_context(tc.tile_pool(name="weights", bufs=1))
        wih_sb = wpool.tile([128, KD, G], bf16, tag="wih")
        for k in range(KD):
            nc.sync.dma_start(wih_sb[:, k, :], wihT[k * 128:(k + 1) * 128, :])
        whh_sb = wpool.tile([128, KH, G], fp8, tag="whh")
        for k in range(KH):
            nc.sync.dma_start(whh_sb[:, k, :], whhT[k * 128:(k + 1) * 128, :])
        ident = wpool.tile([128, 128], fp8, tag="ident")
        nc.sync.dma_start(ident[:], identd[:, :])
        bhn = wpool.tile([128, NN, BS], bf16, tag="bhn")
        nc.sync.dma_start(bhn[:], bhnd[:, :, :])
        p1b = wpool.tile([128, MCH], f32, tag="p1b")
        nc.sync.dma_start(p1b[:], p1bd[:, :])
        w1_sb = wpool.tile([128, KH, M], bf16, tag="w1")
        for k in range(KH):
            nc.sync.dma_start(w1_sb[:, k, :], w1T[k * 128:(k + 1) * 128, :])
        w2_sb = wpool.tile([128, MH, A], bf16, tag="w2")
        for k in range(MH):
            nc.sync.dma_start(w2_sb[:, k, :], w2T[k * 128:(k + 1) * 128, :])
        b1_sb = wpool.tile([128, MH], f32, tag="b1")
        nc.sync.dma_start(b1_sb[:], b1T[:, :])
        b2_sb = wpool.tile([A, 1], f32, tag="b2")
        nc.sync.dma_start(b2_sb[:], b2c[:, :])

        xtp = ctx.enter_context(tc.tile_pool(name="xt", bufs=2))
        xpp = ctx.enter_context(tc.tile_pool(name="xp", bufs=2))
        hp = ctx.enter_context(tc.tile_pool(name="h", bufs=2))
        gp = ctx.enter_context(tc.tile_pool(name="gates", bufs=2))
        pp = ctx.enter_context(tc.tile_pool(name="rec_psum", bufs=1, space="PSUM"))
        p1p = ctx.enter_context(tc.tile_pool(name="p1_psum", bufs=2, space="PSUM"))

        def load_xt(c):
            xt = xtp.tile([128, KD, TBC], bf16, tag="xt")
            for k in range(KD):
                nc.sync.dma_start(
                    xt[:, k, :], xTd[k * 128:(k + 1) * 128, c * TBC:(c + 1) * TBC]
                )
            return xt

        def p1_quantum(m, xt_tile, xp_tile):
            ps1 = p1p.tile([128, TBC], f32, tag="p1")
            for k in range(KD):
                nc.tensor.matmul(
                    ps1[:],
                    wih_sb[:, k, m * 128:(m + 1) * 128],
                    xt_tile[:, k, :],
                    start=(k == 0),
                    stop=(k == KD - 1),
                )
            if m % 2 == 0:
                nc.scalar.activation(
                    xp_tile[:, m, :], ps1[:], AF.Identity, bias=p1b[:, m:m + 1]
                )
            else:
                nc.vector.tensor_scalar_add(xp_tile[:, m, :], ps1[:], p1b[:, m:m + 1])

        # Prologue: x_proj chunk 0.
        xt_cur = load_xt(0)
        xp_use = xpp.tile([128, MCH, TBC], bf16, tag="xp")
        for m in range(MCH):
            p1_quantum(m, xt_cur, xp_use)
        xt_next = None
        xp_next = None

        # Recurrence state: h halves.
        h_cur = []
        for p in (0, 1):
            h0 = hp.tile([128, 2, BS], bf16, tag=f"h{p}")
            nc.vector.memset(h0[:], 0.0)
            h_cur.append(h0)

        def rhs_h(k):
            return h_cur[k // 2][:, k % 2, :]

        nsteps_tot = nsteps * rec_mult
        for tt in range(nsteps_tot):
            t = tt % nsteps
            ti = t % CHUNK
            c = t // CHUNK
            if ti == 0:
                if tt > 0:
                    xp_use = xp_next
                xt_next = load_xt((c + 1) % nchunks)
                xp_next = xpp.tile([128, MCH, TBC], bf16, tag="xp")
            sl = slice(ti * BS, (ti + 1) * BS)

            ps_rz = [
                pp.tile([128, 4, BS], f32, tag=f"rz{p}", name=f"psrz{p}")
                for p in (0, 1)
            ]
            ps_n = [
                pp.tile([128, 2, BS], f32, tag=f"n{p}", name=f"psn{p}")
                for p in (0, 1)
            ]

            # identity injections (first write per tile -> start=True)
            for p in (0, 1):
                for i, m in enumerate(RZ_M[p]):
                    nc.tensor.matmul(
                        ps_rz[p][:, i, :], ident[:, :], xp_use[:, m, sl],
                        start=(i == 0), stop=False,
                    )
                for i, m in enumerate(N_M[p]):
                    nc.tensor.matmul(
                        ps_n[p][:, i, :], ident[:, :], bhn[:, m - 8, :],
                        start=(i == 0), stop=False,
                    )
            # W batches, k-major; rz chunks first within each k batch
            for k in range(KH):
                last_k = k == KH - 1
                for p in (0, 1):
                    for i, m in enumerate(RZ_M[p]):
                        nc.tensor.matmul(
                            ps_rz[p][:, i, :],
                            whh_sb[:, k, m * 128:(m + 1) * 128],
                            rhs_h(k),
                            start=False,
                            stop=(last_k and i == 3),
                        )
                for p in (0, 1):
                    for i, m in enumerate(N_M[p]):
                        nc.tensor.matmul(
                            ps_n[p][:, i, :],
                            whh_sb[:, k, m * 128:(m + 1) * 128],
                            rhs_h(k),
                            start=False,
                            stop=(last_k and i == 1),
                        )

            # gate chains per half
            h_new = [None, None]
            for p in (0, 1):
                rz = gp.tile([128, 4, BS], bf16, tag=f"rz{p}", name=f"rz{p}")
                nc.scalar.activation(rz[:], ps_rz[p][:], AF.Sigmoid, scale=INV)
                t1 = gp.tile([128, 2, BS], bf16, tag=f"t1{p}", name=f"t1{p}")
                nc.vector.tensor_mul(t1[:], ps_n[p][:], rz[:, 0:2, :])
                t2 = gp.tile([128, 2, BS], bf16, tag=f"t2{p}", name=f"t2{p}")
                nc.vector.tensor_add(
                    t2[:], t1[:], xp_use[:, 8 + 2 * p:10 + 2 * p, sl]
                )
                zb = gp.tile([128, 2, BS], bf16, tag=f"zb{p}", name=f"zb{p}")
                nc.vector.tensor_scalar(
                    zb[:], rz[:, 2:4, :], -1.0, 1.0, op0=ALU.mult, op1=ALU.add
                )
                m1 = gp.tile([128, 2, BS], bf16, tag=f"m1{p}", name=f"m1{p}")
                nc.vector.tensor_mul(m1[:], rz[:, 2:4, :], h_cur[p][:])
                ng = gp.tile([128, 2, BS], bf16, tag=f"n{p}", name=f"n{p}")
                nc.scalar.activation(ng[:], t2[:], AF.Tanh, scale=INV)
                m2 = gp.tile([128, 2, BS], bf16, tag=f"m2{p}", name=f"m2{p}")
                nc.vector.tensor_mul(m2[:], zb[:], ng[:])
                hn = hp.tile([128, 2, BS], bf16, tag=f"h{p}", name=f"hn{p}")
                nc.vector.tensor_add(hn[:], m1[:], m2[:])
                h_new[p] = hn
            h_cur = h_new

            # phase-1 quantum for the next chunk, off the critical path
            if ti < MCH:
                p1_quantum(ti, xt_next, xp_next)

        # Head.
        ps_h = p1p.tile([128, MH, BS], f32, tag="p1")
        for mh in range(MH):
            for k in range(KH):
                nc.tensor.matmul(
                    ps_h[:, mh, :],
                    w1_sb[:, k, mh * 128:(mh + 1) * 128],
                    rhs_h(k),
                    start=(mh == 0 and k == 0),
                    stop=(mh == MH - 1 and k == KH - 1),
                )
        hid = gp.tile([128, MH, BS], bf16, tag="hid")
        for mh in range(MH):
            nc.scalar.activation(
                hid[:, mh, :], ps_h[:, mh, :], AF.Tanh, bias=b1_sb[:, mh:mh + 1]
            )
        ps_a = p1p.tile([A, BS], f32, tag="p1")
        for k in range(MH):
            nc.tensor.matmul(
                ps_a[:],
                w2_sb[:, k, :],
                hid[:, k, :],
                start=(k == 0),
                stop=(k == MH - 1),
            )
        act = gp.tile([A, BS], f32, tag="act")
        nc.scalar.activation(act[:], ps_a[:], AF.Tanh, bias=b2_sb[:, 0:1])
        nc.sync.dma_start(outT[:, :], act[:])

    nc.compile()
    return nc


def prep_inputs(x, W_ih, W_hh, b_ih, b_hh, W1, b1, W2, b2, nsteps: int = T):
    x = np.asarray(x, dtype=np.float32)[:nsteps]
    b_ih = np.asarray(b_ih, np.float32)
    b_hh = np.asarray(b_hh, np.float32)
    whh16 = np.ascontiguousarray(np.asarray(W_hh, np.float32).T) * S
    wih16 = np.ascontiguousarray(np.asarray(W_ih, np.float32).T) * S

    # p1 bias: rz chunks get 16*(b_ih+b_hh); n chunks get 16*b_ih only
    p1b = np.zeros((MCH, 128), np.float32)
    for m in range(MCH):
        sl = slice(m * 128, (m + 1) * 128)
        if m < NRZ:
            p1b[m] = S * (b_ih[sl] + b_hh[sl])
        else:
            p1b[m] = S * b_ih[sl]
    # n-gate hidden bias, broadcast over the PB batch cols
    bhn = np.broadcast_to(
        (S * b_hh[G - NN * 128:].reshape(NN, 128).T)[:, :, None], (128, NN, BS)
    )

    common = {
        "wihT": wih16.astype(bf16_np),
        "whhT": whh16.astype(fp8_np),
        "identd": np.eye(128, dtype=np.float32).astype(fp8_np),
        "bhnd": np.ascontiguousarray(bhn).astype(bf16_np),
        "p1bd": np.ascontiguousarray(p1b.T),
        "w1T": np.ascontiguousarray(np.asarray(W1).T).astype(bf16_np),
        "b1T": np.ascontiguousarray(np.asarray(b1, np.float32).reshape(MH, 128).T),
        "w2T": np.ascontiguousarray(np.asarray(W2).T).astype(bf16_np),
        "b2c": np.ascontiguousarray(np.asarray(b2, np.float32).reshape(A, 1)),
    }
    in_maps = []
    for i in range(NCORES):
        shard = x[:, i * BS:(i + 1) * BS, :].reshape(nsteps * BS, D)
        m = dict(common)
        m["xT"] = np.ascontiguousarray(shard.astype(bf16_np).T)
        in_maps.append(m)
    return in_maps


_CACHE = {}


def run(inputs: dict, nsteps: int = T, trace: bool = False):
    key = nsteps
    if key not in _CACHE:
        _CACHE[key] = build(nsteps)
    nc = _CACHE[key]
    in_maps = prep_inputs(**inputs, nsteps=nsteps)
    res = run_bass_kernel_spmd(
        nc, in_maps, core_ids=list(range(NCORES)), trace=trace
    )
    outs = [r["outT"] for r in res.results]
    full = np.concatenate([o.T for o in outs], axis=0)
    return full.astype(np.float32), res


def kernel(**inputs) -> np.ndarray:
    out, _ = run(inputs)
    return out
